# revision 18
# baseline (speedup 1.0000x reference)
"""Trainium2 Bass kernel for nn_Att_multiply (gnn_message_passing).

reference:
    value[b,i,j,d] = e[b,i,d] * e[b,j,d]                      # [B,N,N,D]
    scores[b,i,j]  = sum_d value[b,i,j,d]*w[d] + b0           # LeakyReLU'd
    alphas         = softmax_j(scores)[..., None]             # [B,N,N,1]
    returns (alphas, value)

Sharding: data-parallel over batch B=32 across 8 cores (4 batches/core);
att_w / att_b replicated.

Per-core strategy (N=D=128):
  - value[b] is produced in layout [i -> partitions, (j,d) -> free] so each
    partition's free row is value[b,i] = 64KiB contiguous in DRAM (ideal DMA
    descriptors; one 2 MiB dma_start per 32-j chunk).
  - The partitions-broadcast operand A[i,(j,d)] = e[b,j,d] is built on the PE
    as outer products ones[128] (x) e_flat_row into PSUM.  fp32 PE matmuls
    lower to slow LOW_HIGH double passes, so instead e is decomposed host-side
    into three bf16 terms (e = hi + lo + lo2, residual ~2^-27 relative) and
    three bf16 matmuls accumulate into fp32 PSUM — ones is exact in bf16.
  - DVE multiplies PSUM A by e[b,i,:] (a 0-step repeat AP of the natural
    e tile) into the staging tile; fp32 tensor_tensor, FD=2048 per op.
  - scores[b] = (e*w) @ e^T via PE (e^T from PE-transpose), LeakyReLU exact
    on DVE, softmax along the free axis (reduce_max / Exp+accum / recip).
"""

import ml_dtypes
import numpy as np

import concourse.bass as bass
import concourse.mybir as mybir
import concourse.tile as tile
from concourse.bass_utils import run_bass_kernel_spmd

N_CORES = 8
B, N, D = 32, 128, 128
BPC = B // N_CORES  # batches per core
F32 = mybir.dt.float32
BF16 = mybir.dt.bfloat16
NEG_SLOPE = 0.01

# value chunking: j axis split into DMA chunks of JCH j's ([128, JCH*D] f32
# staging tile per chunk); each chunk is further split into PSUM tiles of
# JPS j's (JPS*D fp32 = 4 PSUM banks).
JCH = 32
JPS = 16
N_TERMS = 3  # bf16 decomposition terms for the broadcast matmuls


def _legalize_sync(nc: bass.Bass) -> None:
    """Split multi-wait sync_info into standalone EventSemaphore carriers.

    The walrus build here encodes at most one semaphore wait per
    instruction; Tile can attach several (e.g. the kernel-tail drain waits
    on every proc).  A preceding EventSemaphore on the same engine blocks
    identically, so hoist all but one wait onto such carriers.
    """
    counter = [0]

    def make_carrier(engine, wait):
        counter[0] += 1
        return mybir.InstEventSemaphore(
            name=f"antsyncw_{counter[0]}",
            engine=engine,
            ins=[],
            outs=[],
            sync_info=mybir.SyncInfo(on_wait=[wait], on_update=[]),
        )

    for fn in nc.m.functions:
        for blk in fn.blocks:
            if not any(
                inst.sync_info is not None and len(inst.sync_info.on_wait) > 1
                for inst in blk.instructions
            ):
                continue
            new_insts = []
            for inst in blk.instructions:
                si = inst.sync_info
                if si is not None and len(si.on_wait) > 1:
                    waits = list(si.on_wait)
                    for w in waits[:-1]:
                        new_insts.append(make_carrier(inst.engine, w))
                    inst.sync_info = mybir.SyncInfo(
                        on_wait=[waits[-1]], on_update=list(si.on_update)
                    )
                new_insts.append(inst)
            blk.instructions = new_insts


def _rep_free(ap: bass.AP, reps: int) -> bass.AP:
    """Repeat a [P, F] access pattern `reps` times along a new middle free
    dim via a 0-step AP: result reads as [P, reps, F]."""
    assert len(ap.ap) == 2
    return bass.AP(
        tensor=ap.tensor,
        offset=ap.offset,
        ap=[ap.ap[0], [0, reps], ap.ap[1]],
    )


def build_nc(legalize: bool = True) -> bass.Bass:
    nc = bass.Bass()
    e_dram = nc.dram_tensor("embeddings", [BPC, N, D], F32, kind="ExternalInput")
    ehi_dram = nc.dram_tensor("e_hi", [BPC, N, D], BF16, kind="ExternalInput")
    elo_dram = nc.dram_tensor("e_lo", [BPC, N, D], BF16, kind="ExternalInput")
    elo2_dram = nc.dram_tensor("e_lo2", [BPC, N, D], BF16, kind="ExternalInput")
    w_dram = nc.dram_tensor("att_w", [D], F32, kind="ExternalInput")
    b_dram = nc.dram_tensor("att_b", [1], F32, kind="ExternalInput")
    alphas_dram = nc.dram_tensor("alphas", [BPC, N, N], F32, kind="ExternalOutput")
    value_dram = nc.dram_tensor("value", [BPC, N, N, D], F32, kind="ExternalOutput")
    eterm_drams = [ehi_dram, elo_dram, elo2_dram][:N_TERMS]

    with tile.TileContext(nc) as tc:
        with (
            tc.tile_pool(name="singles", bufs=1) as singles,
            tc.tile_pool(name="eload", bufs=2) as eload,
            tc.tile_pool(name="att", bufs=2) as att,
            tc.tile_pool(name="outp", bufs=4) as outp,
            tc.tile_pool(name="ps", bufs=2, space=bass.MemorySpace.PSUM) as ps,
        ):
            # all-ones stationary matrix for the broadcast matmuls: K=128 so
            # the whole systolic array stays busy (HAM un-throttles to
            # 2.4 GHz; bf16 128-col weights get the fast weight load).  The
            # rhs term tiles are zero except partitions {0,32,64,96}, which
            # hold e rows; the ones-weighted column sums reduce to a copy.
            ones_bf = singles.tile([N, N], BF16)
            nc.vector.memset(ones_bf, 1.0)
            # persistent sparse staging tiles, one per bf16 term: layout
            # S[32*jj, s*512 + jj*128 + d] = e_term[b, 4*s + jj, d];
            # zeroed once, only the data runs are rewritten per batch.
            stage = [
                singles.tile([N, (N // 4) * 512], BF16,
                             name=f"stage{t}", tag=f"stage{t}")
                for t in range(N_TERMS)
            ]
            for st in stage:
                nc.gpsimd.memset(st, 0.0)
            ident = singles.tile([N, N], F32)
            nc.vector.memset(ident, 1.0)
            nc.gpsimd.affine_select(
                ident, ident, [[-1, N]], mybir.AluOpType.is_equal, 0.0,
                channel_multiplier=1,
            )
            w_col = singles.tile([D, 1], F32)
            nc.sync.dma_start(out=w_col, in_=w_dram.rearrange("(d one) -> d one", one=1))
            bias_col = singles.tile([N, 1], F32)
            nc.gpsimd.dma_start(out=bias_col, in_=b_dram[:].to_broadcast((N, 1)))

            for b in range(BPC):
                e_nat = eload.tile([N, D], F32, tag="e_nat")
                nc.sync.dma_start(out=e_nat, in_=e_dram[b])
                # restage the bf16 terms into the sparse tiles: partition
                # 32*jj gets e rows jj, jj+4, jj+8, ... as 128-elem runs at
                # free offset s*512 + jj*128
                for t, tdram in enumerate(eterm_drams):
                    src = tdram[b].rearrange("(s four) d -> four s d", four=4)
                    for jj in range(4):
                        dst = stage[t][32 * jj:32 * jj + 1, :].rearrange(
                            "p (s x) -> p s x", x=512
                        )[:, :, jj * D:(jj + 1) * D]
                        nc.sync.dma_start(out=dst, in_=src[jj:jj + 1])

                # ---------- alphas[b] ----------
                aps_att = ps.tile([N, JPS * D], F32, tag="a_ps")
                eT_ps = aps_att[:, 0:128]
                nc.tensor.transpose(eT_ps, e_nat, ident)
                eT = att.tile([D, N], F32, tag="eT")
                nc.vector.tensor_copy(eT, eT_ps)
                ewT = att.tile([D, N], F32, tag="ewT")
                nc.vector.tensor_scalar_mul(ewT, eT, w_col)
                sc_ps = aps_att[:, 512:640]
                nc.tensor.matmul(sc_ps, ewT, eT)  # scores[i,j] = sum_d ew[i,d] e[j,d]
                # LeakyReLU(x + bias) = max(x+b, slope*(x+b)), exact in fp32
                xb = att.tile([N, N], F32, tag="xb")
                nc.vector.tensor_scalar_add(xb, sc_ps, bias_col)
                xs = att.tile([N, N], F32, tag="xs")
                nc.scalar.mul(xs, xb, NEG_SLOPE)
                sc = att.tile([N, N], F32, tag="sc")
                nc.vector.tensor_max(sc, xb, xs)
                # softmax along free (j) axis
                nmax = att.tile([N, 1], F32, tag="nmax")
                nc.vector.tensor_reduce(
                    nmax, sc, axis=mybir.AxisListType.X, op=mybir.AluOpType.max,
                    negate=True,
                )
                pexp = att.tile([N, N], F32, tag="pexp")
                ssum = att.tile([N, 1], F32, tag="ssum")
                nc.scalar.activation(
                    pexp, sc, mybir.ActivationFunctionType.Exp,
                    bias=nmax, scale=1.0, accum_out=ssum,
                )
                rinv = att.tile([N, 1], F32, tag="rinv")
                nc.vector.reciprocal(rinv, ssum)
                alph = att.tile([N, N], F32, tag="alph")
                nc.vector.tensor_scalar_mul(alph, pexp, rinv)
                nc.sync.dma_start(out=alphas_dram[b], in_=alph)

                # ---------- value[b] ----------
                for c in range(N // JCH):
                    vout = outp.tile([N, JCH * D], F32, tag="vout")
                    for pp in range(JCH // JPS):
                        j0 = c * JCH + pp * JPS  # first j of this PSUM tile
                        a_ps = ps.tile([N, JPS * D], F32, tag="a_ps")
                        for s in range(JPS // 4):  # 512-wide matmul slots
                            g = j0 // 4 + s        # global slot: j's 4g..4g+3
                            for t in range(N_TERMS):
                                nc.tensor.matmul(
                                    a_ps[:, s * 512:(s + 1) * 512],
                                    ones_bf,
                                    stage[t][:, g * 512:(g + 1) * 512],
                                    start=(t == 0),
                                    stop=(t == N_TERMS - 1),
                                )
                        nc.vector.tensor_mul(
                            vout[:, pp * JPS * D:(pp + 1) * JPS * D].rearrange(
                                "i (j d) -> i j d", d=D
                            ),
                            a_ps[:, :].rearrange("i (j d) -> i j d", d=D),
                            _rep_free(e_nat[:, :], JPS),
                        )
                    nc.sync.dma_start(
                        out=value_dram[b, :, c * JCH:(c + 1) * JCH, :].rearrange(
                            "i j d -> i (j d)"
                        ),
                        in_=vout,
                    )
    if legalize:
        _legalize_sync(nc)
    return nc


_NC_CACHE = None


def _get_nc() -> bass.Bass:
    global _NC_CACHE
    if _NC_CACHE is None:
        _NC_CACHE = build_nc()
    return _NC_CACHE


def _decompose_bf16(e: np.ndarray):
    """e (fp32) ~= hi + lo + lo2, each bf16; residual ~2^-27 relative."""
    hi = e.astype(ml_dtypes.bfloat16)
    r = e - hi.astype(np.float32)
    lo = r.astype(ml_dtypes.bfloat16)
    r2 = r - lo.astype(np.float32)
    lo2 = r2.astype(ml_dtypes.bfloat16)
    return hi, lo, lo2


def make_in_maps(embeddings, att_w, att_b):
    embeddings = np.ascontiguousarray(embeddings, dtype=np.float32)
    att_w = np.ascontiguousarray(att_w, dtype=np.float32)
    att_b = np.ascontiguousarray(att_b, dtype=np.float32)
    assert embeddings.shape == (B, N, D)

    hi, lo, lo2 = _decompose_bf16(embeddings)
    terms = {"e_hi": hi, "e_lo": lo, "e_lo2": lo2}

    in_maps = []
    for c in range(N_CORES):
        sl = slice(c * BPC, (c + 1) * BPC)
        m = {
            "embeddings": embeddings[sl],
            "att_w": att_w,
            "att_b": att_b,
        }
        for name in list(terms)[:N_TERMS]:
            m[name] = np.ascontiguousarray(terms[name][sl])
        in_maps.append(m)
    return in_maps


def kernel(embeddings: np.ndarray, att_w: np.ndarray, att_b: np.ndarray, **_):
    nc = _get_nc()
    in_maps = make_in_maps(embeddings, att_w, att_b)
    res = run_bass_kernel_spmd(nc, in_maps, core_ids=list(range(N_CORES)))
    alphas = np.concatenate([r["alphas"] for r in res.results], axis=0)
    value = np.concatenate([r["value"] for r in res.results], axis=0)
    return alphas[..., None], value


# revision 19
# speedup vs baseline: 1.1330x; 1.1330x over previous
"""Trainium2 Bass kernel for nn_Att_multiply (gnn_message_passing).

reference:
    value[b,i,j,d] = e[b,i,d] * e[b,j,d]                      # [B,N,N,D]
    scores[b,i,j]  = sum_d value[b,i,j,d]*w[d] + b0           # LeakyReLU'd
    alphas         = softmax_j(scores)[..., None]             # [B,N,N,1]
    returns (alphas, value)

Sharding: data-parallel over batch B=32 across 8 cores (4 batches/core);
att_w / att_b replicated.

Per-core strategy (N=D=128):
  - value[b] is produced in layout [i -> partitions, (j,d) -> free] so each
    partition's free row is value[b,i] = 64KiB contiguous in DRAM (ideal DMA
    descriptors; one 2 MiB dma_start per 32-j chunk).
  - The partitions-broadcast operand A[i,(j,d)] = e[b,j,d] is built on the PE
    as outer products ones[128] (x) e_flat_row into PSUM.  fp32 PE matmuls
    lower to slow LOW_HIGH double passes, so instead e is decomposed host-side
    into three bf16 terms (e = hi + lo + lo2, residual ~2^-27 relative) and
    three bf16 matmuls accumulate into fp32 PSUM — ones is exact in bf16.
  - DVE multiplies PSUM A by e[b,i,:] (a 0-step repeat AP of the natural
    e tile) into the staging tile; fp32 tensor_tensor, FD=2048 per op.
  - scores[b] = (e*w) @ e^T via PE (e^T from PE-transpose), LeakyReLU exact
    on DVE, softmax along the free axis (reduce_max / Exp+accum / recip).
"""

import ml_dtypes
import numpy as np

import concourse.bass as bass
import concourse.mybir as mybir
import concourse.tile as tile
from concourse.bass_utils import run_bass_kernel_spmd

N_CORES = 8
B, N, D = 32, 128, 128
BPC = B // N_CORES  # batches per core
F32 = mybir.dt.float32
BF16 = mybir.dt.bfloat16
NEG_SLOPE = 0.01

# value chunking: j axis split into DMA chunks of JCH j's ([128, JCH*D] f32
# staging tile per chunk); each chunk is further split into PSUM tiles of
# JPS j's (JPS*D fp32 = 4 PSUM banks).
JCH = 32
JPS = 16
N_TERMS = 3  # bf16 decomposition terms for the broadcast matmuls


def _legalize_sync(nc: bass.Bass) -> None:
    """Split multi-wait sync_info into standalone EventSemaphore carriers.

    The walrus build here encodes at most one semaphore wait per
    instruction; Tile can attach several (e.g. the kernel-tail drain waits
    on every proc).  A preceding EventSemaphore on the same engine blocks
    identically, so hoist all but one wait onto such carriers.
    """
    counter = [0]

    def make_carrier(engine, wait):
        counter[0] += 1
        return mybir.InstEventSemaphore(
            name=f"antsyncw_{counter[0]}",
            engine=engine,
            ins=[],
            outs=[],
            sync_info=mybir.SyncInfo(on_wait=[wait], on_update=[]),
        )

    for fn in nc.m.functions:
        for blk in fn.blocks:
            if not any(
                inst.sync_info is not None and len(inst.sync_info.on_wait) > 1
                for inst in blk.instructions
            ):
                continue
            new_insts = []
            for inst in blk.instructions:
                si = inst.sync_info
                if si is not None and len(si.on_wait) > 1:
                    waits = list(si.on_wait)
                    for w in waits[:-1]:
                        new_insts.append(make_carrier(inst.engine, w))
                    inst.sync_info = mybir.SyncInfo(
                        on_wait=[waits[-1]], on_update=list(si.on_update)
                    )
                new_insts.append(inst)
            blk.instructions = new_insts


def _rep_free(ap: bass.AP, reps: int) -> bass.AP:
    """Repeat a [P, F] access pattern `reps` times along a new middle free
    dim via a 0-step AP: result reads as [P, reps, F]."""
    assert len(ap.ap) == 2
    return bass.AP(
        tensor=ap.tensor,
        offset=ap.offset,
        ap=[ap.ap[0], [0, reps], ap.ap[1]],
    )


def build_nc(legalize: bool = True) -> bass.Bass:
    nc = bass.Bass()
    e_dram = nc.dram_tensor("embeddings", [BPC, N, D], F32, kind="ExternalInput")
    ehi_dram = nc.dram_tensor("e_hi", [BPC, N, D], BF16, kind="ExternalInput")
    elo_dram = nc.dram_tensor("e_lo", [BPC, N, D], BF16, kind="ExternalInput")
    elo2_dram = nc.dram_tensor("e_lo2", [BPC, N, D], BF16, kind="ExternalInput")
    w_dram = nc.dram_tensor("att_w", [D], F32, kind="ExternalInput")
    b_dram = nc.dram_tensor("att_b", [1], F32, kind="ExternalInput")
    alphas_dram = nc.dram_tensor("alphas", [BPC, N, N], F32, kind="ExternalOutput")
    value_dram = nc.dram_tensor("value", [BPC, N, N, D], F32, kind="ExternalOutput")
    eterm_drams = [ehi_dram, elo_dram, elo2_dram][:N_TERMS]

    with tile.TileContext(nc) as tc:
        with (
            tc.tile_pool(name="singles", bufs=1) as singles,
            tc.tile_pool(name="eload", bufs=2) as eload,
            tc.tile_pool(name="att", bufs=2) as att,
            tc.tile_pool(name="outp", bufs=4) as outp,
            tc.tile_pool(name="ps", bufs=2, space=bass.MemorySpace.PSUM) as ps,
        ):
            # all-ones stationary matrix for the broadcast matmuls: K=128 so
            # the whole systolic array stays busy (HAM un-throttles to
            # 2.4 GHz; bf16 128-col weights get the fast weight load).  The
            # rhs term tiles are zero except partitions {0,32,64,96}, which
            # hold e rows; the ones-weighted column sums reduce to a copy.
            ones_bf = singles.tile([N, N], BF16)
            nc.vector.memset(ones_bf, 1.0)
            # persistent sparse staging tiles, one per bf16 term: layout
            # S[32*jj, s*512 + jj*128 + d] = e_term[b, 4*s + jj, d];
            # zeroed once, only the data runs are rewritten per batch.
            stage = [
                singles.tile([N, (N // 4) * 512], BF16,
                             name=f"stage{t}", tag=f"stage{t}")
                for t in range(N_TERMS)
            ]
            for t, st in enumerate(stage):
                if t == 1:
                    nc.scalar.memzero(st[:, :])
                else:
                    nc.vector.memset(st, 0.0)
            ident = singles.tile([N, N], F32)
            nc.vector.memset(ident, 1.0)
            nc.gpsimd.affine_select(
                ident, ident, [[-1, N]], mybir.AluOpType.is_equal, 0.0,
                channel_multiplier=1,
            )
            w_col = singles.tile([D, 1], F32)
            nc.sync.dma_start(out=w_col, in_=w_dram.rearrange("(d one) -> d one", one=1))
            bias_col = singles.tile([N, 1], F32)
            nc.gpsimd.dma_start(out=bias_col, in_=b_dram[:].to_broadcast((N, 1)))

            for b in range(BPC):
                e_nat = eload.tile([N, D], F32, tag="e_nat")
                nc.sync.dma_start(out=e_nat, in_=e_dram[b])
                # restage the bf16 terms into the sparse tiles: partition
                # 32*jj gets e rows jj, jj+4, jj+8, ... as 128-elem runs at
                # free offset s*512 + jj*128
                for t, tdram in enumerate(eterm_drams):
                    src = tdram[b].rearrange("(s four) d -> four s d", four=4)
                    for jj in range(4):
                        dst = stage[t][32 * jj:32 * jj + 1, :].rearrange(
                            "p (s x) -> p s x", x=512
                        )[:, :, jj * D:(jj + 1) * D]
                        nc.sync.dma_start(out=dst, in_=src[jj:jj + 1])

                # ---------- alphas[b] ----------
                aps_att = ps.tile([N, JPS * D], F32, tag="a_ps")
                eT_ps = aps_att[:, 0:128]
                nc.tensor.transpose(eT_ps, e_nat, ident)
                eT = att.tile([D, N], F32, tag="eT")
                nc.vector.tensor_copy(eT, eT_ps)
                ewT = att.tile([D, N], F32, tag="ewT")
                nc.vector.tensor_scalar_mul(ewT, eT, w_col)
                sc_ps = aps_att[:, 512:640]
                nc.tensor.matmul(sc_ps, ewT, eT)  # scores[i,j] = sum_d ew[i,d] e[j,d]
                # LeakyReLU(x + bias) = max(x+b, slope*(x+b)), exact in fp32
                xb = att.tile([N, N], F32, tag="xb")
                nc.vector.tensor_scalar_add(xb, sc_ps, bias_col)
                xs = att.tile([N, N], F32, tag="xs")
                nc.scalar.mul(xs, xb, NEG_SLOPE)
                sc = att.tile([N, N], F32, tag="sc")
                nc.vector.tensor_max(sc, xb, xs)
                # softmax along free (j) axis
                nmax = att.tile([N, 1], F32, tag="nmax")
                nc.vector.tensor_reduce(
                    nmax, sc, axis=mybir.AxisListType.X, op=mybir.AluOpType.max,
                    negate=True,
                )
                pexp = att.tile([N, N], F32, tag="pexp")
                ssum = att.tile([N, 1], F32, tag="ssum")
                nc.scalar.activation(
                    pexp, sc, mybir.ActivationFunctionType.Exp,
                    bias=nmax, scale=1.0, accum_out=ssum,
                )
                rinv = att.tile([N, 1], F32, tag="rinv")
                nc.vector.reciprocal(rinv, ssum)
                alph = att.tile([N, N], F32, tag="alph")
                nc.vector.tensor_scalar_mul(alph, pexp, rinv)
                nc.sync.dma_start(out=alphas_dram[b], in_=alph)

                # ---------- value[b] ----------
                for c in range(N // JCH):
                    vout = outp.tile([N, JCH * D], F32, tag="vout")
                    for pp in range(JCH // JPS):
                        j0 = c * JCH + pp * JPS  # first j of this PSUM tile
                        a_ps = ps.tile([N, JPS * D], F32, tag="a_ps")
                        for s in range(JPS // 4):  # 512-wide matmul slots
                            g = j0 // 4 + s        # global slot: j's 4g..4g+3
                            for t in range(N_TERMS):
                                nc.tensor.matmul(
                                    a_ps[:, s * 512:(s + 1) * 512],
                                    ones_bf,
                                    stage[t][:, g * 512:(g + 1) * 512],
                                    start=(t == 0),
                                    stop=(t == N_TERMS - 1),
                                )
                        nc.vector.tensor_mul(
                            vout[:, pp * JPS * D:(pp + 1) * JPS * D].rearrange(
                                "i (j d) -> i j d", d=D
                            ),
                            a_ps[:, :].rearrange("i (j d) -> i j d", d=D),
                            _rep_free(e_nat[:, :], JPS),
                        )
                    nc.sync.dma_start(
                        out=value_dram[b, :, c * JCH:(c + 1) * JCH, :].rearrange(
                            "i j d -> i (j d)"
                        ),
                        in_=vout,
                    )
    if legalize:
        _legalize_sync(nc)
    return nc


_NC_CACHE = None


def _get_nc() -> bass.Bass:
    global _NC_CACHE
    if _NC_CACHE is None:
        _NC_CACHE = build_nc()
    return _NC_CACHE


def _decompose_bf16(e: np.ndarray):
    """e (fp32) ~= hi + lo + lo2, each bf16; residual ~2^-27 relative."""
    hi = e.astype(ml_dtypes.bfloat16)
    r = e - hi.astype(np.float32)
    lo = r.astype(ml_dtypes.bfloat16)
    r2 = r - lo.astype(np.float32)
    lo2 = r2.astype(ml_dtypes.bfloat16)
    return hi, lo, lo2


def make_in_maps(embeddings, att_w, att_b):
    embeddings = np.ascontiguousarray(embeddings, dtype=np.float32)
    att_w = np.ascontiguousarray(att_w, dtype=np.float32)
    att_b = np.ascontiguousarray(att_b, dtype=np.float32)
    assert embeddings.shape == (B, N, D)

    hi, lo, lo2 = _decompose_bf16(embeddings)
    terms = {"e_hi": hi, "e_lo": lo, "e_lo2": lo2}

    in_maps = []
    for c in range(N_CORES):
        sl = slice(c * BPC, (c + 1) * BPC)
        m = {
            "embeddings": embeddings[sl],
            "att_w": att_w,
            "att_b": att_b,
        }
        for name in list(terms)[:N_TERMS]:
            m[name] = np.ascontiguousarray(terms[name][sl])
        in_maps.append(m)
    return in_maps


def kernel(embeddings: np.ndarray, att_w: np.ndarray, att_b: np.ndarray, **_):
    nc = _get_nc()
    in_maps = make_in_maps(embeddings, att_w, att_b)
    res = run_bass_kernel_spmd(nc, in_maps, core_ids=list(range(N_CORES)))
    alphas = np.concatenate([r["alphas"] for r in res.results], axis=0)
    value = np.concatenate([r["value"] for r in res.results], axis=0)
    return alphas[..., None], value
